# revision 10
# baseline (speedup 1.0000x reference)
"""Chamfer distance (B=4, N1=N2=8192, D=3) on 8 NeuronCores.

Sharding: core = b*2 + h handles xyz1[b, h*4096:(h+1)*4096] vs all of xyz2[b].

Per-core device kernel:
  - Host lifts points to K=5 vectors so a single fp32 matmul produces
    NEGATED squared distances: -d[i,j] = -|x_i|^2 - |y_j|^2 + 2 x_i.y_j.
    lhsT[:, i] = [-|x_i|^2, 1, 2x_i0, 2x_i1, 2x_i2]   (stationary, i on partitions)
    rhs[:, j]  = [1, -|y_j|^2, y_j0, y_j1, y_j2]      (moving, j on free axis)
  - With negated distances every min becomes a max:
      dist1[i] = -max_j(-d) : elementwise TT-max over j-groups into rowacc,
                 then one free-axis reduce per 128-row block.
      dist2[j] = -max_i(-d) : elementwise TT-max over i-blocks into colacc,
                 then one partition_all_reduce(max) per column group at the end.
  - PSUM egress is load-balanced: ACT copies tiles to fp16 SBUF (enabling DVE
    2x-mode TTs and GPSIMD TTs), DVE handles the rest directly from PSUM.
"""

import os
import numpy as np

B, N1, N2, D = 4, 8192, 8192, 3
N_CORES = 8
I_PER_CORE = N1 // 2          # 4096 xyz1 rows per core
J = N2                        # 8192 xyz2 points (full)
IB = I_PER_CORE // 128        # 32 i-blocks
GW = 1024                     # PSUM group width (2 banks, 2 matmuls)
NG = J // GW                  # 8 column groups per i-block
NEG_INF_F16 = -60000.0

# Per-group reduction path, len NG. 'A': DVE fp32 TTs straight from PSUM (no
# copy). 'B': ACT copy to fp16, DVE fp16 TTs. 'C': ACT copy to fp16, DVE fp16
# row TT, GPSIMD fp16 col TT.  Group 0 must be 'B' or 'C' (its ACT copy
# initializes rowacc).
PATHS = list(os.environ.get("CHAMFER_PATHS", "BBBBBBBB"))

_CACHE = {}


def _build_program():
    from contextlib import ExitStack

    import concourse.bacc as bacc
    import concourse.tile as tile
    from concourse import mybir
    from concourse import bass_isa

    f32 = mybir.dt.float32
    f16 = mybir.dt.float16
    MAX = mybir.AluOpType.max

    nc = bacc.Bacc("TRN2")
    # lifted1 and lifted2 concatenated along columns: one DMA -> the first
    # matmul only needs a single sync wait (walrus LDWEIGHTS limit).
    l_d = nc.declare_dram_parameter("lifted", [5, I_PER_CORE + J], f32, isOutput=False)
    d1_d = nc.declare_dram_parameter("d1out", [128, IB], f32, isOutput=True)
    d2_d = nc.declare_dram_parameter("d2out", [1, J], f16, isOutput=True)

    with tile.TileContext(nc) as tc, ExitStack() as ctx:
        const = ctx.enter_context(tc.tile_pool(name="const", bufs=1))
        psum = ctx.enter_context(tc.tile_pool(name="psum", bufs=3, space="PSUM"))
        cpool = ctx.enter_context(tc.tile_pool(name="copies", bufs=6))
        rpool = ctx.enter_context(tc.tile_pool(name="rowacc", bufs=2))
        fpool = ctx.enter_context(tc.tile_pool(name="fold", bufs=2))

        lsb = const.tile([5, I_PER_CORE + J], f32, tag="lifted")
        nc.sync.dma_start(lsb[:], l_d[:])
        l1sb = lsb[:, 0:I_PER_CORE]

        d1sb = const.tile([128, IB], f32, tag="d1sb")

        colacc = []
        for gc in range(NG):
            t = const.tile([128, GW], f16, tag=f"colacc{gc}")
            nc.vector.memset(t[:], NEG_INF_F16)
            colacc.append(t)

        for ib in range(IB):
            lhsT = l1sb[:, ib * 128:(ib + 1) * 128]
            rowacc = rpool.tile([128, GW], f16, tag="rowacc")
            for gc in range(NG):
                path = PATHS[gc]
                pt = psum.tile([128, GW], f32, tag="pt")
                for h in range(GW // 512):
                    jlo = I_PER_CORE + gc * GW + h * 512
                    nc.tensor.matmul(
                        pt[:, h * 512:(h + 1) * 512],
                        lhsT,
                        lsb[:, jlo:jlo + 512],
                        start=True,
                        stop=True,
                    )
                if path == "A":
                    # fp32 TTs directly from PSUM (fp16 accumulator out)
                    if gc == 0:
                        nc.vector.tensor_copy(rowacc[:], pt[:])
                    else:
                        nc.vector.tensor_tensor(rowacc[:], rowacc[:], pt[:], op=MAX)
                    nc.vector.tensor_tensor(colacc[gc][:], colacc[gc][:], pt[:], op=MAX)
                else:
                    if gc == 0:
                        cp = rowacc  # ACT copy doubles as rowacc init
                    else:
                        cp = cpool.tile([128, GW], f16, tag="cp")
                    nc.scalar.copy(cp[:], pt[:])
                    if gc != 0:
                        nc.vector.tensor_tensor(rowacc[:], rowacc[:], cp[:], op=MAX)
                    col_eng = nc.gpsimd if path == "C" else nc.vector
                    col_eng.tensor_tensor(colacc[gc][:], colacc[gc][:], cp[:], op=MAX)
            # fold rowacc [128, GW] -> d1sb[:, ib]
            nc.vector.tensor_tensor(
                rowacc[:, 0:512], rowacc[:, 0:512], rowacc[:, 512:GW], op=MAX
            )
            nc.vector.tensor_reduce(
                d1sb[:, ib:ib + 1], rowacc[:, 0:512], axis=mybir.AxisListType.X, op=MAX
            )

        nc.sync.dma_start(d1_d[:], d1sb[:])

        for gc in range(NG):
            fold = fpool.tile([128, GW], f16, tag="fold")
            nc.gpsimd.partition_all_reduce(
                fold[:], colacc[gc][:], 128, bass_isa.ReduceOp.max
            )
            nc.sync.dma_start(d2_d[0:1, gc * GW:(gc + 1) * GW], fold[0:1, :])

    nc.compile()
    return nc


def _get_program():
    if "nc" not in _CACHE:
        _CACHE["nc"] = _build_program()
    return _CACHE["nc"]


def _lift(xyz1_half, xyz2_full):
    """Pack [lifted1 | lifted2] into one [5, n1+n2] f32 array."""
    x1 = np.ascontiguousarray(xyz1_half, dtype=np.float32)
    x2 = np.ascontiguousarray(xyz2_full, dtype=np.float32)
    sq1 = (x1 * x1).sum(-1)
    sq2 = (x2 * x2).sum(-1)
    n1 = x1.shape[0]
    n2 = x2.shape[0]
    lifted = np.empty((5, n1 + n2), np.float32)
    lifted[0, :n1] = -sq1
    lifted[1, :n1] = 1.0
    lifted[2:5, :n1] = 2.0 * x1.T
    lifted[0, n1:] = 1.0
    lifted[1, n1:] = -sq2
    lifted[2:5, n1:] = x2.T
    return lifted


def kernel(xyz1, xyz2):
    from concourse.bass_utils import run_bass_kernel_spmd

    xyz1 = np.asarray(xyz1, dtype=np.float32)
    xyz2 = np.asarray(xyz2, dtype=np.float32)

    nc = _get_program()
    in_maps = []
    for core in range(N_CORES):
        b, h = divmod(core, 2)
        lifted = _lift(xyz1[b, h * I_PER_CORE:(h + 1) * I_PER_CORE], xyz2[b])
        in_maps.append({"lifted": lifted})

    trace = bool(int(os.environ.get("CHAMFER_TRACE", "0")))
    out = run_bass_kernel_spmd(nc, in_maps, list(range(N_CORES)), trace=trace)
    _CACHE["last_exec_ns"] = out.exec_time_ns
    _CACHE["last_results"] = out
    res = out.results

    d1_sum = 0.0
    d2_sum = 0.0
    for b in range(B):
        for h in range(2):
            m1 = res[b * 2 + h]["d1out"]  # [128, IB], max_j of -d
            d1_sum += -np.float64(m1.T.sum())
        m2a = res[b * 2 + 0]["d2out"][0].astype(np.float32)  # [J], max over half i
        m2b = res[b * 2 + 1]["d2out"][0].astype(np.float32)
        d2_sum += -np.float64(np.maximum(m2a, m2b).sum())

    mean1 = d1_sum / (B * N1)
    mean2 = d2_sum / (B * N2)
    return np.float32(mean1 + mean2)


# revision 15
# speedup vs baseline: 2.5169x; 2.5169x over previous
"""Chamfer distance (B=4, N1=N2=8192, D=3) on 8 NeuronCores.

Sharding: core = b*2 + h handles xyz1[b, h*4096:(h+1)*4096] vs all of xyz2[b].

Per-core device kernel:
  - Host lifts points to K=5 vectors so a single fp32 matmul produces
    NEGATED squared distances: -d[i,j] = -|x_i|^2 - |y_j|^2 + 2 x_i.y_j.
    lhsT[:, i] = [-|x_i|^2, 1, 2x_i0, 2x_i1, 2x_i2]   (stationary, i on partitions)
    rhs[:, j]  = [1, -|y_j|^2, y_j0, y_j1, y_j2]      (moving, j on free axis)
  - With negated distances every min becomes a max:
      dist1[i] = -max_j(-d) : elementwise TT-max over j-groups into rowacc,
                 then one free-axis reduce per 128-row block.
      dist2[j] = -max_i(-d) : elementwise TT-max over i-blocks into colacc,
                 then one partition_all_reduce(max) per column group at the end.
  - PSUM egress is load-balanced: ACT copies tiles to fp16 SBUF (enabling DVE
    2x-mode TTs and GPSIMD TTs), DVE handles the rest directly from PSUM.
"""

import os
import numpy as np

B, N1, N2, D = 4, 8192, 8192, 3
N_CORES = 8
I_PER_CORE = N1 // 2          # 4096 xyz1 rows per core
J = N2                        # 8192 xyz2 points (full)
IB = I_PER_CORE // 128        # 32 i-blocks
GW = 1024                     # PSUM group width (2 banks, 2 matmuls)
NG = J // GW                  # 8 column groups per i-block
NEG_INF_F16 = -60000.0

# Per-group reduction path, len NG. 'A': DVE fp32 TTs straight from PSUM (no
# copy). 'B': ACT copy to fp16, DVE fp16 TTs. 'C': ACT copy to fp16, DVE fp16
# row TT, GPSIMD fp16 col TT.  Group 0 must be 'B' or 'C' (its ACT copy
# initializes rowacc).
PATHS = list(os.environ.get("CHAMFER_PATHS", "BBBBBBBB"))

_CACHE = {}


def _build_program():
    from contextlib import ExitStack

    import concourse.bacc as bacc
    import concourse.tile as tile
    from concourse import mybir
    from concourse import bass_isa

    f32 = mybir.dt.float32
    f16 = mybir.dt.float16
    bf16 = mybir.dt.bfloat16
    MAX = mybir.AluOpType.max

    nc = bacc.Bacc("TRN2")
    # lifted1 and lifted2 concatenated along columns: one DMA -> the first
    # matmul only needs a single sync wait (walrus LDWEIGHTS limit).
    # K=16 bf16 hi/lo split-precision rows (fp32 matmul runs at 1/4 rate on
    # the PE; bf16 streams 2 cols/cycle and K<=128 is free).
    l_d = nc.declare_dram_parameter("lifted", [16, I_PER_CORE + J], bf16, isOutput=False)
    d1_d = nc.declare_dram_parameter("d1out", [128, IB], f32, isOutput=True)
    d2_d = nc.declare_dram_parameter("d2out", [1, J], f16, isOutput=True)

    with tile.TileContext(nc) as tc, ExitStack() as ctx:
        const = ctx.enter_context(tc.tile_pool(name="const", bufs=1))
        psum = ctx.enter_context(tc.tile_pool(name="psum", bufs=3, space="PSUM"))
        cpool = ctx.enter_context(tc.tile_pool(name="copies", bufs=6))
        rpool = ctx.enter_context(tc.tile_pool(name="rowacc", bufs=2))
        fpool = ctx.enter_context(tc.tile_pool(name="fold", bufs=2))

        lsb = const.tile([16, I_PER_CORE + J], bf16, tag="lifted")
        nc.sync.dma_start(lsb[:], l_d[:])
        l1sb = lsb[:, 0:I_PER_CORE]

        d1sb = const.tile([128, IB], f32, tag="d1sb")

        colacc = []
        for gc in range(NG):
            t = const.tile([128, GW], f16, tag=f"colacc{gc}")
            nc.vector.memset(t[:], NEG_INF_F16)
            colacc.append(t)

        for ib in range(IB):
            lhsT = l1sb[:, ib * 128:(ib + 1) * 128]
            rowacc = rpool.tile([128, GW], f16, tag="rowacc")
            for gc in range(NG):
                path = PATHS[gc]
                pt = psum.tile([128, GW], f32, tag="pt")
                for h in range(GW // 512):
                    jlo = I_PER_CORE + gc * GW + h * 512
                    nc.tensor.matmul(
                        pt[:, h * 512:(h + 1) * 512],
                        lhsT,
                        lsb[:, jlo:jlo + 512],
                        start=True,
                        stop=True,
                    )
                if path == "A":
                    # fp32 TTs directly from PSUM (fp16 accumulator out)
                    if gc == 0:
                        nc.vector.tensor_copy(rowacc[:], pt[:])
                    else:
                        nc.vector.tensor_tensor(rowacc[:], rowacc[:], pt[:], op=MAX)
                    nc.vector.tensor_tensor(colacc[gc][:], colacc[gc][:], pt[:], op=MAX)
                else:
                    if gc == 0:
                        cp = rowacc  # ACT copy doubles as rowacc init
                    else:
                        cp = cpool.tile([128, GW], f16, tag="cp")
                    nc.scalar.copy(cp[:], pt[:])
                    if gc != 0:
                        nc.vector.tensor_tensor(rowacc[:], rowacc[:], cp[:], op=MAX)
                    col_eng = nc.gpsimd if path == "C" else nc.vector
                    col_eng.tensor_tensor(colacc[gc][:], colacc[gc][:], cp[:], op=MAX)
            # fold rowacc [128, GW] -> d1sb[:, ib]
            nc.vector.tensor_tensor(
                rowacc[:, 0:512], rowacc[:, 0:512], rowacc[:, 512:GW], op=MAX
            )
            nc.vector.tensor_reduce(
                d1sb[:, ib:ib + 1], rowacc[:, 0:512], axis=mybir.AxisListType.X, op=MAX
            )

        nc.sync.dma_start(d1_d[:], d1sb[:])

        for gc in range(NG):
            fold = fpool.tile([128, GW], f16, tag="fold")
            nc.gpsimd.partition_all_reduce(
                fold[:], colacc[gc][:], 128, bass_isa.ReduceOp.max
            )
            nc.sync.dma_start(d2_d[0:1, gc * GW:(gc + 1) * GW], fold[0:1, :])

    nc.compile()
    return nc


def _get_program():
    if "nc" not in _CACHE:
        _CACHE["nc"] = _build_program()
    return _CACHE["nc"]


def _bf16_split(v):
    import ml_dtypes

    bf16 = ml_dtypes.bfloat16
    hi = v.astype(bf16).astype(np.float32)
    lo = (v - hi).astype(bf16).astype(np.float32)
    return hi, lo


def _lift(xyz1_half, xyz2_full):
    """Pack [lifted1 | lifted2] into one [16, n1+n2] bf16 array.

    -d[i,j] = -sq1_i - sq2_j + (2*x_i).y_j, every fp32 factor split into
    bf16 hi+lo so each of the 16 k-rows is an exact-ish bf16 product pair.
    """
    import ml_dtypes

    x1 = np.ascontiguousarray(xyz1_half, dtype=np.float32)
    x2 = np.ascontiguousarray(xyz2_full, dtype=np.float32)
    sq1 = (x1 * x1).sum(-1)
    sq2 = (x2 * x2).sum(-1)
    n1 = x1.shape[0]
    n2 = x2.shape[0]
    A = np.empty((16, n1), np.float32)
    B = np.empty((16, n2), np.float32)
    A[0], A[1] = _bf16_split(-sq1)
    B[0] = 1.0
    B[1] = 1.0
    A[2] = 1.0
    A[3] = 1.0
    B[2], B[3] = _bf16_split(-sq2)
    for d in range(3):
        ah, al = _bf16_split(2.0 * x1[:, d])
        bh, bl = _bf16_split(x2[:, d])
        r = 4 + 4 * d
        A[r + 0], B[r + 0] = ah, bh
        A[r + 1], B[r + 1] = ah, bl
        A[r + 2], B[r + 2] = al, bh
        A[r + 3], B[r + 3] = al, bl
    lifted = np.empty((16, n1 + n2), ml_dtypes.bfloat16)
    lifted[:, :n1] = A
    lifted[:, n1:] = B
    return lifted


def kernel(xyz1, xyz2):
    from concourse.bass_utils import run_bass_kernel_spmd

    xyz1 = np.asarray(xyz1, dtype=np.float32)
    xyz2 = np.asarray(xyz2, dtype=np.float32)

    nc = _get_program()
    in_maps = []
    for core in range(N_CORES):
        b, h = divmod(core, 2)
        lifted = _lift(xyz1[b, h * I_PER_CORE:(h + 1) * I_PER_CORE], xyz2[b])
        in_maps.append({"lifted": lifted})

    trace = bool(int(os.environ.get("CHAMFER_TRACE", "0")))
    out = run_bass_kernel_spmd(nc, in_maps, list(range(N_CORES)), trace=trace)
    _CACHE["last_exec_ns"] = out.exec_time_ns
    _CACHE["last_results"] = out
    res = out.results

    d1_sum = 0.0
    d2_sum = 0.0
    for b in range(B):
        for h in range(2):
            m1 = res[b * 2 + h]["d1out"]  # [128, IB], max_j of -d
            d1_sum += -np.float64(m1.T.sum())
        m2a = res[b * 2 + 0]["d2out"][0].astype(np.float32)  # [J], max over half i
        m2b = res[b * 2 + 1]["d2out"][0].astype(np.float32)
        d2_sum += -np.float64(np.maximum(m2a, m2b).sum())

    mean1 = d1_sum / (B * N1)
    mean2 = d2_sum / (B * N2)
    return np.float32(mean1 + mean2)


# revision 22
# speedup vs baseline: 2.5606x; 1.0174x over previous
"""Chamfer distance (B=4, N1=N2=8192, D=3) on 8 NeuronCores.

Sharding: core = b*2 + h handles xyz1[b, h*4096:(h+1)*4096] vs all of xyz2[b].

Per-core device kernel:
  - Host lifts points to K=5 vectors so a single fp32 matmul produces
    NEGATED squared distances: -d[i,j] = -|x_i|^2 - |y_j|^2 + 2 x_i.y_j.
    lhsT[:, i] = [-|x_i|^2, 1, 2x_i0, 2x_i1, 2x_i2]   (stationary, i on partitions)
    rhs[:, j]  = [1, -|y_j|^2, y_j0, y_j1, y_j2]      (moving, j on free axis)
  - With negated distances every min becomes a max:
      dist1[i] = -max_j(-d) : elementwise TT-max over j-groups into rowacc,
                 then one free-axis reduce per 128-row block.
      dist2[j] = -max_i(-d) : elementwise TT-max over i-blocks into colacc,
                 then one partition_all_reduce(max) per column group at the end.
  - PSUM egress is load-balanced: ACT copies tiles to fp16 SBUF (enabling DVE
    2x-mode TTs and GPSIMD TTs), DVE handles the rest directly from PSUM.
"""

import os
import numpy as np

B, N1, N2, D = 4, 8192, 8192, 3
N_CORES = 8
I_PER_CORE = N1 // 2          # 4096 xyz1 rows per core
J = N2                        # 8192 xyz2 points (full)
IB = I_PER_CORE // 128        # 32 i-blocks
GW = 2048                     # PSUM group width (4 banks, 4 matmuls)
NG = J // GW                  # 4 column groups per i-block
KDIM = 24                     # bf16 3-way-split lifted contraction depth
NEG_INF_F16 = -60000.0

# Per-group reduction path, len NG. 'A': DVE fp32 TTs straight from PSUM (no
# copy). 'B': ACT copy to fp16, DVE fp16 TTs. 'C': ACT copy to fp16, DVE fp16
# row TT, GPSIMD fp16 col TT.  Group 0 must be 'B' or 'C' (its ACT copy
# initializes rowacc).
PATHS = list(os.environ.get("CHAMFER_PATHS", "BBBB"))

_CACHE = {}


def _build_program():
    from contextlib import ExitStack

    import concourse.bacc as bacc
    import concourse.tile as tile
    from concourse import mybir
    from concourse import bass_isa

    f32 = mybir.dt.float32
    f16 = mybir.dt.float16
    bf16 = mybir.dt.bfloat16
    MAX = mybir.AluOpType.max

    nc = bacc.Bacc("TRN2")
    # lifted1 and lifted2 concatenated along columns: one DMA -> the first
    # matmul only needs a single sync wait (walrus LDWEIGHTS limit).
    # K=24 bf16 3-way-split rows (fp32 matmul runs at 1/4 rate on the PE;
    # bf16 streams 2 cols/cycle and K<=128 is free).
    l_d = nc.declare_dram_parameter(
        "lifted", [KDIM, I_PER_CORE + J], bf16, isOutput=False
    )
    d1_d = nc.declare_dram_parameter("d1out", [128, IB], f32, isOutput=True)
    d2_d = nc.declare_dram_parameter("d2out", [1, J], f16, isOutput=True)

    with tile.TileContext(nc) as tc, ExitStack() as ctx:
        const = ctx.enter_context(tc.tile_pool(name="const", bufs=1))
        psum = ctx.enter_context(tc.tile_pool(name="psum", bufs=2, space="PSUM"))
        cpool = ctx.enter_context(tc.tile_pool(name="copies", bufs=6))
        rpool = ctx.enter_context(tc.tile_pool(name="rowacc", bufs=2))
        fpool = ctx.enter_context(tc.tile_pool(name="fold", bufs=2))

        lsb = const.tile([KDIM, I_PER_CORE + J], bf16, tag="lifted")
        nc.sync.dma_start(lsb[:], l_d[:])
        l1sb = lsb[:, 0:I_PER_CORE]

        d1sb = const.tile([128, IB], f32, tag="d1sb")

        colacc = []
        for gc in range(NG):
            t = const.tile([128, GW], f16, tag=f"colacc{gc}")
            nc.vector.memset(t[:], NEG_INF_F16)
            colacc.append(t)

        for ib in range(IB):
            lhsT = l1sb[:, ib * 128:(ib + 1) * 128]
            rowacc = rpool.tile([128, GW], f16, tag="rowacc")
            for gc in range(NG):
                path = PATHS[gc]
                pt = psum.tile([128, GW], f32, tag="pt")
                for h in range(GW // 512):
                    jlo = I_PER_CORE + gc * GW + h * 512
                    nc.tensor.matmul(
                        pt[:, h * 512:(h + 1) * 512],
                        lhsT,
                        lsb[:, jlo:jlo + 512],
                        start=True,
                        stop=True,
                    )
                if path == "A":
                    # fp32 TTs directly from PSUM (fp16 accumulator out)
                    if gc == 0:
                        nc.vector.tensor_copy(rowacc[:], pt[:])
                    else:
                        nc.vector.tensor_tensor(rowacc[:], rowacc[:], pt[:], op=MAX)
                    nc.vector.tensor_tensor(colacc[gc][:], colacc[gc][:], pt[:], op=MAX)
                else:
                    if gc == 0:
                        cp = rowacc  # ACT copy doubles as rowacc init
                    else:
                        cp = cpool.tile([128, GW], f16, tag="cp")
                    nc.scalar.copy(cp[:], pt[:])
                    if gc != 0:
                        nc.vector.tensor_tensor(rowacc[:], rowacc[:], cp[:], op=MAX)
                    col_eng = nc.gpsimd if path == "C" else nc.vector
                    col_eng.tensor_tensor(colacc[gc][:], colacc[gc][:], cp[:], op=MAX)
            # fold rowacc [128, GW] -> d1sb[:, ib]
            w = GW
            while w > 512:
                w //= 2
                nc.vector.tensor_tensor(
                    rowacc[:, 0:w], rowacc[:, 0:w], rowacc[:, w:2 * w], op=MAX
                )
            nc.vector.tensor_reduce(
                d1sb[:, ib:ib + 1], rowacc[:, 0:w], axis=mybir.AxisListType.X, op=MAX
            )

        nc.sync.dma_start(d1_d[:], d1sb[:])

        for gc in range(NG):
            fold = fpool.tile([128, GW], f16, tag="fold")
            nc.gpsimd.partition_all_reduce(
                fold[:], colacc[gc][:], 128, bass_isa.ReduceOp.max
            )
            nc.sync.dma_start(d2_d[0:1, gc * GW:(gc + 1) * GW], fold[0:1, :])

    nc.compile()
    return nc


def _get_program():
    if "nc" not in _CACHE:
        _CACHE["nc"] = _build_program()
    return _CACHE["nc"]


def _bf16_split3(v):
    import ml_dtypes

    bf16 = ml_dtypes.bfloat16
    hi = v.astype(bf16).astype(np.float32)
    r = v - hi
    mid = r.astype(bf16).astype(np.float32)
    lo = (r - mid).astype(bf16).astype(np.float32)
    return hi, mid, lo


def _lift(xyz1_half, xyz2_full):
    """Pack [lifted1 | lifted2] into one [24, n1+n2] bf16 array.

    -d[i,j] = -sq1_i - sq2_j + (2*x_i).y_j, every fp32 factor split 3-way
    into bf16 (hi, mid, lo); product pairs keep all terms down to ~2^-27:
    hh, hm, mh, hl, lh, mm per coordinate.
    """
    import ml_dtypes

    x1 = np.ascontiguousarray(xyz1_half, dtype=np.float32)
    x2 = np.ascontiguousarray(xyz2_full, dtype=np.float32)
    sq1 = (x1 * x1).sum(-1)
    sq2 = (x2 * x2).sum(-1)
    n1 = x1.shape[0]
    n2 = x2.shape[0]
    A = np.empty((KDIM, n1), np.float32)
    B = np.empty((KDIM, n2), np.float32)
    A[0], A[1], A[2] = _bf16_split3(-sq1)
    B[0:3] = 1.0
    A[3:6] = 1.0
    B[3], B[4], B[5] = _bf16_split3(-sq2)
    for d in range(3):
        ah, am, al = _bf16_split3(2.0 * x1[:, d])
        bh, bm, bl = _bf16_split3(x2[:, d])
        r = 6 + 6 * d
        A[r + 0], B[r + 0] = ah, bh
        A[r + 1], B[r + 1] = ah, bm
        A[r + 2], B[r + 2] = am, bh
        A[r + 3], B[r + 3] = ah, bl
        A[r + 4], B[r + 4] = al, bh
        A[r + 5], B[r + 5] = am, bm
    lifted = np.empty((KDIM, n1 + n2), ml_dtypes.bfloat16)
    lifted[:, :n1] = A
    lifted[:, n1:] = B
    return lifted


def kernel(xyz1, xyz2):
    from concourse.bass_utils import run_bass_kernel_spmd

    xyz1 = np.asarray(xyz1, dtype=np.float32)
    xyz2 = np.asarray(xyz2, dtype=np.float32)

    nc = _get_program()
    in_maps = []
    for core in range(N_CORES):
        b, h = divmod(core, 2)
        lifted = _lift(xyz1[b, h * I_PER_CORE:(h + 1) * I_PER_CORE], xyz2[b])
        in_maps.append({"lifted": lifted})

    trace = bool(int(os.environ.get("CHAMFER_TRACE", "0")))
    out = run_bass_kernel_spmd(nc, in_maps, list(range(N_CORES)), trace=trace)
    _CACHE["last_exec_ns"] = out.exec_time_ns
    _CACHE["last_results"] = out
    res = out.results

    d1_sum = 0.0
    d2_sum = 0.0
    for b in range(B):
        for h in range(2):
            m1 = res[b * 2 + h]["d1out"]  # [128, IB], max_j of -d
            d1_sum += -np.float64(m1.T.sum())
        m2a = res[b * 2 + 0]["d2out"][0].astype(np.float32)  # [J], max over half i
        m2b = res[b * 2 + 1]["d2out"][0].astype(np.float32)
        d2_sum += -np.float64(np.maximum(m2a, m2b).sum())

    mean1 = d1_sum / (B * N1)
    mean2 = d2_sum / (B * N2)
    return np.float32(mean1 + mean2)


# revision 28
# speedup vs baseline: 2.5975x; 1.0144x over previous
"""Chamfer distance (B=4, N1=N2=8192, D=3) on 8 NeuronCores.

Sharding: core = b*2 + h handles xyz1[b, h*4096:(h+1)*4096] vs all of xyz2[b].

Per-core device kernel:
  - Host lifts points to K=24 bf16 vectors (3-way hi/mid/lo split per fp32
    factor) so a single bf16 matmul produces NEGATED squared distances in
    PSUM: -d[i,j] = -|x_i|^2 - |y_j|^2 + (2x_i).y_j, accurate to ~2^-27.
  - K=24 <= 32, so the PE runs in 32x128 row-tiling mode: 4 concurrent
    matmuls (tile_position (32g, 0)) fill a 4-bank PSUM group [128, 2048]
    in about one matmul's time. The lifted operands are replicated at SBUF
    partition offsets 0/32/64/96 to feed the four row-groups.
  - With negated distances every min becomes a max:
      dist1[i]: elementwise TT-max over j-groups into rowacc[128, 2048],
                folded with DMA-accum + one tensor_reduce per 128-row block.
      dist2[j]: elementwise TT-max over i-blocks into colacc[gc], folded by
                gpsimd partition_all_reduce(max) at the end.
  - PSUM egress: ACT copies each group to fp16 SBUF; DVE (2x fp16 TTs) and
    SWDGE DMA-accumulate (accum_op=max) split the reduction passes.
"""

import os
import numpy as np

B, N1, N2, D = 4, 8192, 8192, 3
N_CORES = 8
I_PER_CORE = N1 // 2          # 4096 xyz1 rows per core
J = N2                        # 8192 xyz2 points (full)
IB = I_PER_CORE // 128        # 32 i-blocks
GW = 2048                     # PSUM group width (4 banks, 4 packed matmuls)
NG = J // GW                  # 4 column groups per i-block
KDIM = 24                     # bf16 3-way-split lifted contraction depth
NEG_INF_F16 = -60000.0

# Column-pass engine per group: 'V' = DVE fp16 TT ('D' DMA-accum max is
# rejected by walrus: DMACopy only supports add-style cce ops).
COL_ENG = list(os.environ.get("CHAMFER_COL", "VVVV"))
# Row accumulation: 'T' = tensor_tensor_reduce (fused accum+reduce, no fold),
# 'V' = tensor_tensor + explicit fold.
ROW_MODE = os.environ.get("CHAMFER_ROW", "V")

_CACHE = {}


def _build_program():
    from contextlib import ExitStack

    import concourse.bacc as bacc
    import concourse.tile as tile
    from concourse import mybir
    from concourse import bass_isa

    f32 = mybir.dt.float32
    f16 = mybir.dt.float16
    bf16 = mybir.dt.bfloat16
    MAX = mybir.AluOpType.max

    nc = bacc.Bacc("TRN2", num_swdge_queues=2)
    # Lifted operands for all four PE row-groups: partitions 32g+k (k<24)
    # hold lifted row k. Split into two tensors so the two DMAs overlap.
    l1_d = nc.declare_dram_parameter("lifted1", [128, I_PER_CORE], bf16, isOutput=False)
    l2_d = nc.declare_dram_parameter("lifted2", [128, J], bf16, isOutput=False)
    d1_d = nc.declare_dram_parameter("d1out", [128, IB], f32, isOutput=True)
    d2_d = nc.declare_dram_parameter("d2out", [1, J], f16, isOutput=True)

    with tile.TileContext(nc) as tc, ExitStack() as ctx:
        const = ctx.enter_context(tc.tile_pool(name="const", bufs=1))
        psum = ctx.enter_context(tc.tile_pool(name="psum", bufs=2, space="PSUM"))
        cpool = ctx.enter_context(tc.tile_pool(name="copies", bufs=6))
        rpool = ctx.enter_context(tc.tile_pool(name="rowacc", bufs=2))
        fpool = ctx.enter_context(tc.tile_pool(name="fold", bufs=2))

        l1sb = const.tile([128, I_PER_CORE], bf16, tag="lifted1")
        l2sb = const.tile([128, J], bf16, tag="lifted2")
        nc.sync.dma_start(l1sb[:], l1_d[:])
        nc.sync.dma_start(l2sb[:], l2_d[:])

        d1sb = const.tile([128, IB], f32, tag="d1sb")
        neginf = const.tile([128, 1], f32, tag="neginf")
        nc.gpsimd.memset(neginf[:], NEG_INF_F16)

        colacc = []
        for gc in range(NG):
            t = const.tile([128, GW], f16, tag=f"colacc{gc}")
            nc.gpsimd.memset(t[:], NEG_INF_F16)
            colacc.append(t)

        for ib in range(IB):
            rowacc = rpool.tile([128, GW], f16, tag="rowacc")
            rp = rpool.tile([128, NG], f32, tag="rowpart")
            for gc in range(NG):
                pt = psum.tile([128, GW], f32, tag="pt")
                for g in range(4):
                    jlo = gc * GW + g * 512
                    nc.tensor.matmul(
                        pt[:, g * 512:(g + 1) * 512],
                        l1sb[32 * g:32 * g + KDIM, ib * 128:(ib + 1) * 128],
                        l2sb[32 * g:32 * g + KDIM, jlo:jlo + 512],
                        start=True,
                        stop=True,
                        tile_position=(32 * g, 0),
                    )
                if gc == 0:
                    cp = rowacc  # ACT copy doubles as rowacc init
                else:
                    cp = cpool.tile([128, GW], f16, tag="cp")
                nc.scalar.copy(cp[:], pt[:])
                if ROW_MODE == "T":
                    # fused: rowacc = max(rowacc-or-cp, cp); rp = running
                    # free-axis max (the gc=NG-1 call yields dist1 directly)
                    nc.vector.tensor_tensor_reduce(
                        out=rowacc[:] if gc != 0 else cp[:],
                        in0=rowacc[:] if gc != 0 else cp[:],
                        in1=cp[:],
                        scale=1.0,
                        scalar=neginf[:, 0:1] if gc == 0 else rp[:, gc - 1:gc],
                        op0=MAX,
                        op1=MAX,
                        accum_out=(
                            d1sb[:, ib:ib + 1] if gc == NG - 1 else rp[:, gc:gc + 1]
                        ),
                    )
                elif gc != 0:
                    nc.vector.tensor_tensor(rowacc[:], rowacc[:], cp[:], op=MAX)
                nc.vector.tensor_tensor(colacc[gc][:], colacc[gc][:], cp[:], op=MAX)
            if ROW_MODE != "T":
                # fold rowacc [128, GW] -> d1sb[:, ib]
                w = GW
                while w > 512:
                    w //= 2
                    nc.vector.tensor_tensor(
                        rowacc[:, 0:w], rowacc[:, 0:w], rowacc[:, w:2 * w], op=MAX
                    )
                nc.vector.tensor_reduce(
                    d1sb[:, ib:ib + 1], rowacc[:, 0:w],
                    axis=mybir.AxisListType.X, op=MAX,
                )

        nc.sync.dma_start(d1_d[:], d1sb[:])

        for gc in range(NG):
            fold = fpool.tile([128, GW], f16, tag="fold")
            nc.gpsimd.partition_all_reduce(
                fold[:], colacc[gc][:], 128, bass_isa.ReduceOp.max
            )
            nc.sync.dma_start(d2_d[0:1, gc * GW:(gc + 1) * GW], fold[0:1, :])

    nc.compile()
    return nc


def _get_program():
    if "nc" not in _CACHE:
        _CACHE["nc"] = _build_program()
    return _CACHE["nc"]


def _bf16_split3(v):
    import ml_dtypes

    bf16 = ml_dtypes.bfloat16
    hi = v.astype(bf16).astype(np.float32)
    r = v - hi
    mid = r.astype(bf16).astype(np.float32)
    lo = (r - mid).astype(bf16).astype(np.float32)
    return hi, mid, lo


def _lift(xyz1_half, xyz2_full):
    """Pack [lifted1 | lifted2] into one [128, n1+n2] bf16 array, the 24
    lifted rows replicated at partition offsets 0/32/64/96 for the four PE
    row-groups.

    -d[i,j] = -sq1_i - sq2_j + (2*x_i).y_j, every fp32 factor split 3-way
    into bf16 (hi, mid, lo); product pairs keep all terms down to ~2^-27:
    hh, hm, mh, hl, lh, mm per coordinate.
    """
    import ml_dtypes

    x1 = np.ascontiguousarray(xyz1_half, dtype=np.float32)
    x2 = np.ascontiguousarray(xyz2_full, dtype=np.float32)
    sq1 = (x1 * x1).sum(-1)
    sq2 = (x2 * x2).sum(-1)
    n1 = x1.shape[0]
    n2 = x2.shape[0]
    A = np.empty((KDIM, n1), np.float32)
    B_ = np.empty((KDIM, n2), np.float32)
    A[0], A[1], A[2] = _bf16_split3(-sq1)
    B_[0:3] = 1.0
    A[3:6] = 1.0
    B_[3], B_[4], B_[5] = _bf16_split3(-sq2)
    for d in range(3):
        ah, am, al = _bf16_split3(2.0 * x1[:, d])
        bh, bm, bl = _bf16_split3(x2[:, d])
        r = 6 + 6 * d
        A[r + 0], B_[r + 0] = ah, bh
        A[r + 1], B_[r + 1] = ah, bm
        A[r + 2], B_[r + 2] = am, bh
        A[r + 3], B_[r + 3] = ah, bl
        A[r + 4], B_[r + 4] = al, bh
        A[r + 5], B_[r + 5] = am, bm
    lifted1 = np.zeros((128, n1), ml_dtypes.bfloat16)
    lifted2 = np.zeros((128, n2), ml_dtypes.bfloat16)
    for g in range(4):
        lifted1[32 * g:32 * g + KDIM] = A
        lifted2[32 * g:32 * g + KDIM] = B_
    return lifted1, lifted2


def kernel(xyz1, xyz2):
    from concourse.bass_utils import run_bass_kernel_spmd

    xyz1 = np.asarray(xyz1, dtype=np.float32)
    xyz2 = np.asarray(xyz2, dtype=np.float32)

    nc = _get_program()
    in_maps = []
    for core in range(N_CORES):
        b, h = divmod(core, 2)
        l1, l2 = _lift(xyz1[b, h * I_PER_CORE:(h + 1) * I_PER_CORE], xyz2[b])
        in_maps.append({"lifted1": l1, "lifted2": l2})

    trace = bool(int(os.environ.get("CHAMFER_TRACE", "0")))
    out = run_bass_kernel_spmd(nc, in_maps, list(range(N_CORES)), trace=trace)
    _CACHE["last_exec_ns"] = out.exec_time_ns
    _CACHE["last_results"] = out
    res = out.results

    d1_sum = 0.0
    d2_sum = 0.0
    for b in range(B):
        for h in range(2):
            m1 = res[b * 2 + h]["d1out"]  # [128, IB], max_j of -d
            d1_sum += -np.float64(m1.T.sum())
        m2a = res[b * 2 + 0]["d2out"][0].astype(np.float32)  # [J], max over half i
        m2b = res[b * 2 + 1]["d2out"][0].astype(np.float32)
        d2_sum += -np.float64(np.maximum(m2a, m2b).sum())

    mean1 = d1_sum / (B * N1)
    mean2 = d2_sum / (B * N2)
    return np.float32(mean1 + mean2)


# revision 32
# speedup vs baseline: 2.6168x; 1.0074x over previous
"""Chamfer distance (B=4, N1=N2=8192, D=3) on 8 NeuronCores.

Sharding: core = b*2 + h handles xyz1[b, h*4096:(h+1)*4096] vs all of xyz2[b].

Per-core device kernel:
  - Host lifts points to K=24 bf16 vectors (3-way hi/mid/lo split per fp32
    factor) so a single bf16 matmul produces NEGATED squared distances in
    PSUM: -d[i,j] = -|x_i|^2 - |y_j|^2 + (2x_i).y_j, accurate to ~2^-27.
  - K=24 <= 32, so the PE runs in 32x128 row-tiling mode: 4 concurrent
    matmuls (tile_position (32g, 0)) fill a 4-bank PSUM group [128, 2048]
    in about one matmul's time. The lifted operands are replicated at SBUF
    partition offsets 0/32/64/96 to feed the four row-groups.
  - With negated distances every min becomes a max:
      dist1[i]: elementwise TT-max over j-groups into rowacc[128, 2048],
                folded with DMA-accum + one tensor_reduce per 128-row block.
      dist2[j]: elementwise TT-max over i-blocks into colacc[gc], folded by
                gpsimd partition_all_reduce(max) at the end.
  - PSUM egress: ACT copies each group to fp16 SBUF; DVE (2x fp16 TTs) and
    SWDGE DMA-accumulate (accum_op=max) split the reduction passes.
"""

import os
import numpy as np

B, N1, N2, D = 4, 8192, 8192, 3
N_CORES = 8
I_PER_CORE = N1 // 2          # 4096 xyz1 rows per core
J = N2                        # 8192 xyz2 points (full)
IB = I_PER_CORE // 128        # 32 i-blocks
GW = 2048                     # PSUM group width (4 banks, 4 packed matmuls)
NG = J // GW                  # 4 column groups per i-block
KDIM = 24                     # bf16 3-way-split lifted contraction depth
NEG_INF_F16 = -60000.0

# Column-pass engine per group: 'V' = DVE fp16 TT ('D' DMA-accum max is
# rejected by walrus: DMACopy only supports add-style cce ops).
COL_ENG = list(os.environ.get("CHAMFER_COL", "VVVV"))
# Row accumulation: 'T' = tensor_tensor_reduce (fused accum+reduce, no fold),
# 'V' = tensor_tensor + explicit fold.
ROW_MODE = os.environ.get("CHAMFER_ROW", "V")

_CACHE = {}


def _build_program():
    from contextlib import ExitStack

    import concourse.bacc as bacc
    import concourse.tile as tile
    from concourse import mybir
    from concourse import bass_isa

    f32 = mybir.dt.float32
    f16 = mybir.dt.float16
    bf16 = mybir.dt.bfloat16
    MAX = mybir.AluOpType.max

    nc = bacc.Bacc("TRN2", num_swdge_queues=2)
    # Lifted operands for all four PE row-groups: partitions 32g+k (k<24)
    # hold lifted row k. Split into two tensors so the two DMAs overlap.
    l1_d = nc.declare_dram_parameter("lifted1", [128, I_PER_CORE], bf16, isOutput=False)
    l2_d = nc.declare_dram_parameter("lifted2", [128, J], bf16, isOutput=False)
    d1_d = nc.declare_dram_parameter("d1out", [128, IB], f32, isOutput=True)
    d2_d = nc.declare_dram_parameter("d2out", [1, J], f16, isOutput=True)

    with tile.TileContext(nc) as tc, ExitStack() as ctx:
        const = ctx.enter_context(tc.tile_pool(name="const", bufs=1))
        psum = ctx.enter_context(tc.tile_pool(name="psum", bufs=2, space="PSUM"))
        cpool = ctx.enter_context(tc.tile_pool(name="copies", bufs=6))
        rpool = ctx.enter_context(tc.tile_pool(name="rowacc", bufs=2))
        fpool = ctx.enter_context(tc.tile_pool(name="fold", bufs=2))

        l1sb = const.tile([128, I_PER_CORE], bf16, tag="lifted1")
        l2sb = const.tile([128, J], bf16, tag="lifted2")
        nc.sync.dma_start(l1sb[:], l1_d[:])
        # chunked so the first matmuls only wait on their own slice
        for c in range(NG):
            nc.sync.dma_start(
                l2sb[:, c * GW:(c + 1) * GW], l2_d[:, c * GW:(c + 1) * GW]
            )

        d1sb = const.tile([128, IB], f32, tag="d1sb")
        neginf = const.tile([128, 1], f32, tag="neginf")
        nc.gpsimd.memset(neginf[:], NEG_INF_F16)

        colacc = []
        for gc in range(NG):
            t = const.tile([128, GW], f16, tag=f"colacc{gc}")
            nc.gpsimd.memset(t[:], NEG_INF_F16)
            colacc.append(t)

        for ib in range(IB):
            rowacc = rpool.tile([128, GW], f16, tag="rowacc")
            if ROW_MODE == "T":
                rp = rpool.tile([128, NG], f32, tag="rowpart")
            for gc in range(NG):
                pt = psum.tile([128, GW], f32, tag="pt")
                for g in range(4):
                    jlo = gc * GW + g * 512
                    nc.tensor.matmul(
                        pt[:, g * 512:(g + 1) * 512],
                        l1sb[32 * g:32 * g + KDIM, ib * 128:(ib + 1) * 128],
                        l2sb[32 * g:32 * g + KDIM, jlo:jlo + 512],
                        start=True,
                        stop=True,
                        tile_position=(32 * g, 0),
                    )
                if gc == 0:
                    cp = rowacc  # ACT copy doubles as rowacc init
                else:
                    cp = cpool.tile([128, GW], f16, tag="cp")
                nc.scalar.copy(cp[:], pt[:])
                if ROW_MODE == "T" and gc != 0:
                    # fused: rowacc' = max(rowacc, cp) with free-axis max as a
                    # side product; rowacc already includes group 0 (the ACT
                    # copy init), so the gc=NG-1 accum IS dist1 for this block.
                    rowacc_new = rpool.tile([128, GW], f16, tag="rowacc")
                    nc.vector.tensor_tensor_reduce(
                        out=rowacc_new[:],
                        in0=rowacc[:],
                        in1=cp[:],
                        scale=1.0,
                        scalar=float(NEG_INF_F16),
                        op0=MAX,
                        op1=MAX,
                        accum_out=(
                            d1sb[:, ib:ib + 1] if gc == NG - 1 else rp[:, gc:gc + 1]
                        ),
                    )
                    rowacc = rowacc_new
                elif gc != 0:
                    nc.vector.tensor_tensor(rowacc[:], rowacc[:], cp[:], op=MAX)
                nc.vector.tensor_tensor(colacc[gc][:], colacc[gc][:], cp[:], op=MAX)
            if ROW_MODE != "T":
                # fold rowacc [128, GW] -> d1sb[:, ib]
                w = GW
                while w > 512:
                    w //= 2
                    nc.vector.tensor_tensor(
                        rowacc[:, 0:w], rowacc[:, 0:w], rowacc[:, w:2 * w], op=MAX
                    )
                nc.vector.tensor_reduce(
                    d1sb[:, ib:ib + 1], rowacc[:, 0:w],
                    axis=mybir.AxisListType.X, op=MAX,
                )

        nc.sync.dma_start(d1_d[:], d1sb[:])

        for gc in range(NG):
            fold = fpool.tile([128, GW], f16, tag="fold")
            nc.gpsimd.partition_all_reduce(
                fold[:], colacc[gc][:], 128, bass_isa.ReduceOp.max
            )
            nc.sync.dma_start(d2_d[0:1, gc * GW:(gc + 1) * GW], fold[0:1, :])

    nc.compile()
    return nc


def _get_program():
    if "nc" not in _CACHE:
        _CACHE["nc"] = _build_program()
    return _CACHE["nc"]


def _bf16_split3(v):
    import ml_dtypes

    bf16 = ml_dtypes.bfloat16
    hi = v.astype(bf16).astype(np.float32)
    r = v - hi
    mid = r.astype(bf16).astype(np.float32)
    lo = (r - mid).astype(bf16).astype(np.float32)
    return hi, mid, lo


def _lift(xyz1_half, xyz2_full):
    """Pack [lifted1 | lifted2] into one [128, n1+n2] bf16 array, the 24
    lifted rows replicated at partition offsets 0/32/64/96 for the four PE
    row-groups.

    -d[i,j] = -sq1_i - sq2_j + (2*x_i).y_j, every fp32 factor split 3-way
    into bf16 (hi, mid, lo); product pairs keep all terms down to ~2^-27:
    hh, hm, mh, hl, lh, mm per coordinate.
    """
    import ml_dtypes

    x1 = np.ascontiguousarray(xyz1_half, dtype=np.float32)
    x2 = np.ascontiguousarray(xyz2_full, dtype=np.float32)
    sq1 = (x1 * x1).sum(-1)
    sq2 = (x2 * x2).sum(-1)
    n1 = x1.shape[0]
    n2 = x2.shape[0]
    A = np.empty((KDIM, n1), np.float32)
    B_ = np.empty((KDIM, n2), np.float32)
    A[0], A[1], A[2] = _bf16_split3(-sq1)
    B_[0:3] = 1.0
    A[3:6] = 1.0
    B_[3], B_[4], B_[5] = _bf16_split3(-sq2)
    for d in range(3):
        ah, am, al = _bf16_split3(2.0 * x1[:, d])
        bh, bm, bl = _bf16_split3(x2[:, d])
        r = 6 + 6 * d
        A[r + 0], B_[r + 0] = ah, bh
        A[r + 1], B_[r + 1] = ah, bm
        A[r + 2], B_[r + 2] = am, bh
        A[r + 3], B_[r + 3] = ah, bl
        A[r + 4], B_[r + 4] = al, bh
        A[r + 5], B_[r + 5] = am, bm
    lifted1 = np.zeros((128, n1), ml_dtypes.bfloat16)
    lifted2 = np.zeros((128, n2), ml_dtypes.bfloat16)
    for g in range(4):
        lifted1[32 * g:32 * g + KDIM] = A
        lifted2[32 * g:32 * g + KDIM] = B_
    return lifted1, lifted2


def kernel(xyz1, xyz2):
    from concourse.bass_utils import run_bass_kernel_spmd

    xyz1 = np.asarray(xyz1, dtype=np.float32)
    xyz2 = np.asarray(xyz2, dtype=np.float32)

    nc = _get_program()
    in_maps = []
    for core in range(N_CORES):
        b, h = divmod(core, 2)
        l1, l2 = _lift(xyz1[b, h * I_PER_CORE:(h + 1) * I_PER_CORE], xyz2[b])
        in_maps.append({"lifted1": l1, "lifted2": l2})

    trace = bool(int(os.environ.get("CHAMFER_TRACE", "0")))
    out = run_bass_kernel_spmd(nc, in_maps, list(range(N_CORES)), trace=trace)
    _CACHE["last_exec_ns"] = out.exec_time_ns
    _CACHE["last_results"] = out
    res = out.results

    d1_sum = 0.0
    d2_sum = 0.0
    for b in range(B):
        for h in range(2):
            m1 = res[b * 2 + h]["d1out"]  # [128, IB], max_j of -d
            d1_sum += -np.float64(m1.T.sum())
        m2a = res[b * 2 + 0]["d2out"][0].astype(np.float32)  # [J], max over half i
        m2b = res[b * 2 + 1]["d2out"][0].astype(np.float32)
        d2_sum += -np.float64(np.maximum(m2a, m2b).sum())

    mean1 = d1_sum / (B * N1)
    mean2 = d2_sum / (B * N2)
    return np.float32(mean1 + mean2)
